# revision 1
# baseline (speedup 1.0000x reference)
"""Grouped-GEMM MoE expert MLP kernel for 8 Trainium2 NeuronCores.

Problem: x [8, 2048, 1024] f32, per-group W1 [8, 4096, 1024], b1 [8, 4096],
W2 [8, 1024, 4096], b2 [8, 1024] (torch Linear convention, y = x @ W.T + b):
  h1 = xg @ W1.T + b1        (per group)
  h2 = h1 @ W2.T + b2
Expert-parallel: core i owns group i entirely — no collectives.

Formulation is fully transposed so every DMA is contiguous and biases land on
the partition axis:
  h1T[o, m]   = matmul(lhsT=W1T[h,o] tiles, rhs=xT[h,m] tiles)  + b1[o]
  outT[ho, m] = matmul(lhsT=W2T[o,ho] tiles, rhs=h1T[o,m] tiles) + b2[ho]
(out = lhsT.T @ rhs contracts the partition axis of both operands.)
Host pre-transposes x/W1/W2 per shard and un-transposes the output.

Matmuls run in float32r (full-rate fp32 mode, 1 cycle/row at free dim 512)
with fp32 PSUM accumulation.

Per-core loop structure: 2 m-chunks of 1024 tokens; inside, 8 o-chunks of 512.
GEMM1 for an o-chunk feeds SBUF tiles h1T; GEMM2 accumulates PSUM over an
o-PAIR (1024, 8 k-steps) then folds into an SBUF accumulator (first pair via
ScalarE copy+bias, later pairs via VectorE add) to keep PSUM pressure at
4+4 banks. Weights are streamed per m-chunk (2 x 33.6 MB), x and out once.
"""
import sys

sys.path.insert(0, "/opt/trn_rl_repo")

import numpy as np

import concourse.bass as bass  # noqa: F401  (bass import initializes mybir deps)
import concourse.mybir as mybir
import concourse.tile as tile
from concourse import bacc
from concourse.bass_utils import run_bass_kernel_spmd

NUM_GEMMS = 8
HIDDEN = 1024
INTER = 4096
M = 2048  # tokens per group

M_CHUNK = 1024  # tokens per chunk (2 chunks)
MS = 512        # matmul moving free dim (fp32 max / one PSUM bank)
O_CHUNK = 512   # GEMM1 / weight-DMA granularity along INTER
O_PAIR = 1024   # GEMM2 PSUM accumulation span along INTER (8 k-steps)

f32 = mybir.dt.float32
f32r = mybir.dt.float32r

N_MC = M // M_CHUNK              # 2
N_PAIR = INTER // O_PAIR         # 4
N_MS = M_CHUNK // MS             # 2
KT1 = HIDDEN // 128              # 8 k-tiles for GEMM1
KT2 = O_PAIR // 128              # 8 k-tiles per GEMM2 psum group
N_OT = O_CHUNK // 128            # 4 o-tiles per o-chunk
N_HT = HIDDEN // 128             # 8 hout-tiles

_NC_CACHE = None


def build_nc():
    """Build + compile the single-core program (same on all 8 cores)."""
    global _NC_CACHE
    if _NC_CACHE is not None:
        return _NC_CACHE

    nc = bacc.Bacc("TRN2", target_bir_lowering=False, debug=False, num_devices=8)
    xT = nc.dram_tensor("xT", [HIDDEN, M], f32r, kind="ExternalInput").ap()
    w1T = nc.dram_tensor("w1T", [HIDDEN, INTER], f32r, kind="ExternalInput").ap()
    b1 = nc.dram_tensor("b1", [128, INTER // 128], f32, kind="ExternalInput").ap()
    w2T = nc.dram_tensor("w2T", [INTER, HIDDEN], f32r, kind="ExternalInput").ap()
    b2 = nc.dram_tensor("b2", [128, HIDDEN // 128], f32, kind="ExternalInput").ap()
    outT = nc.dram_tensor("outT", [HIDDEN, M], f32, kind="ExternalOutput").ap()

    ID = mybir.ActivationFunctionType.Identity

    with tile.TileContext(nc) as tc:
        with (
            tc.tile_pool(name="cst", bufs=1) as cst,
            tc.tile_pool(name="xp", bufs=1) as xp,
            tc.tile_pool(name="hp", bufs=1) as hp,
            tc.tile_pool(name="w1p", bufs=3) as w1p,
            tc.tile_pool(name="w2p", bufs=2) as w2p,
            tc.tile_pool(name="h1p", bufs=2) as h1p,
            tc.tile_pool(name="ps1", bufs=4, space="PSUM") as ps1,
            tc.tile_pool(name="ps2", bufs=4, space="PSUM") as ps2,
        ):
            # PE warmup while the first DMAs fill: releases the HAM clock
            # throttle (4/8 -> 8/8, needs ~3.4us of sustained PE activity)
            # before the real matmuls arrive.
            # Warmup matmuls read a framework const tile (loaded in the
            # preamble, before any DMA can land) broadcast along the free
            # dim; plain fp32 runs at 4 cyc/row so a handful of matmuls
            # spans the ~3.4us HAM un-throttle window.
            ps_junk = ps1.tile([128, MS], f32, tag="ps1", name="ps1t")
            cwarm = nc.const_aps.scalar_like(1.0, ps_junk[:, :])
            cbr = cwarm.broadcast_to([128, MS])
            for _ in range(2):
                nc.tensor.matmul(
                    ps_junk[:1, :], cwarm, cbr, start=True, stop=True,
                )

            b1_sb = cst.tile([128, INTER // 128], f32)
            b2_sb = cst.tile([128, HIDDEN // 128], f32)

            for mc in range(N_MC):
                m0 = mc * M_CHUNK
                # x chunk: [HIDDEN, M_CHUNK] -> [128, KT1 * M_CHUNK].
                # Split per k-tile so the first GEMM1 matmuls can start as
                # soon as k-tile 0 lands (subtile deps) instead of after the
                # whole 4.2 MB chunk.
                xt_sb = xp.tile([128, KT1 * M_CHUNK], f32r, tag="xt")
                xt_dma = []
                for k in range(KT1):
                    xt_dma.append((
                        xt_sb[:, k * M_CHUNK:(k + 1) * M_CHUNK],
                        xT[k * 128:(k + 1) * 128, m0:m0 + M_CHUNK],
                    ))
                if mc != 0:
                    # Non-first chunk: one bulk prefetch queued behind
                    # current work.
                    nc.sync.dma_start(
                        xt_sb[:, :].rearrange("p (a m) -> p a m", m=M_CHUNK),
                        xT[:, m0:m0 + M_CHUNK].rearrange(
                            "(a p) m -> p a m", p=128),
                    )
                    xt_dma = []
                # output accumulator: [HIDDEN, M_CHUNK] -> [128, N_HT * M_CHUNK]
                h2_sb = hp.tile([128, N_HT * M_CHUNK], f32, tag="h2")

                for pair in range(N_PAIR):
                    h1_half = []
                    w2_half = []
                    deferred_w2 = []
                    for half in range(2):
                        oc = pair * 2 + half
                        o0 = oc * O_CHUNK
                        cold = mc == 0 and pair == 0 and half == 0
                        # W1T slice [HIDDEN, O_CHUNK] -> [128, KT1 * O_CHUNK]
                        w1_sb = w1p.tile([128, KT1 * O_CHUNK], f32r, tag="w1")
                        if cold:
                            # Cold fill, ordered to match the ms-outer
                            # consumption order of the first GEMM1 pass.
                            # Scalar's queue clears its preamble ~1.5us
                            # before sync's, so the very first k-tile pair
                            # issues there; x loads split per ms-half so
                            # the ms=0 pass isn't gated on ms=1 bytes.
                            def xt_half(k, ms):
                                return (
                                    xt_sb[:, k * M_CHUNK + ms * MS:
                                          k * M_CHUNK + (ms + 1) * MS],
                                    xT[k * 128:(k + 1) * 128,
                                       m0 + ms * MS:m0 + (ms + 1) * MS],
                                )
                            nc.sync.dma_start(
                                w1_sb[:, 0:O_CHUNK],
                                w1T[0:128, o0:o0 + O_CHUNK],
                            )
                            nc.scalar.dma_start(*xt_half(0, 0))
                            nc.scalar.dma_start(b1_sb[:, :], b1[:, :])
                            nc.scalar.dma_start(b2_sb[:, :], b2[:, :])
                            for k in range(1, KT1):
                                nc.sync.dma_start(
                                    w1_sb[:, k * O_CHUNK:(k + 1) * O_CHUNK],
                                    w1T[k * 128:(k + 1) * 128, o0:o0 + O_CHUNK],
                                )
                                nc.sync.dma_start(*xt_half(k, 0))
                            for k in range(KT1):
                                nc.sync.dma_start(*xt_half(k, 1))
                        else:
                            nc.sync.dma_start(
                                w1_sb[:, :].rearrange("p (a o) -> p a o",
                                                      o=O_CHUNK),
                                w1T[:, o0:o0 + O_CHUNK].rearrange(
                                    "(a p) o -> p a o", p=128),
                            )
                        # The cold half's W2 slice queues here, AFTER this
                        # half's W1 — W1(oc1) is needed ~15us before
                        # W2(oc0), and the sync queue delivers in FIFO
                        # order.
                        for args in deferred_w2:
                            nc.sync.dma_start(*args)
                        deferred_w2 = []

                        # GEMM1: h1T[o0:o0+512, m-chunk]
                        h1_sb = h1p.tile([128, N_OT * M_CHUNK], f32r, tag="h1")
                        if cold:
                            # k-outer order: consume k-tiles as they arrive.
                            # ms outer keeps live PSUM groups at N_OT = 4.
                            for ms in range(N_MS):
                                accs = [ps1.tile([128, MS], f32, tag="ps1",
                                                 name="ps1t")
                                        for _ in range(N_OT)]
                                for k in range(KT1):
                                    for ot in range(N_OT):
                                        nc.tensor.matmul(
                                            accs[ot][:, :],
                                            w1_sb[:, k * O_CHUNK + ot * 128:
                                                  k * O_CHUNK + (ot + 1) * 128],
                                            xt_sb[:, k * M_CHUNK + ms * MS:
                                                  k * M_CHUNK + (ms + 1) * MS],
                                            start=(k == 0),
                                            stop=(k == KT1 - 1),
                                        )
                                for ot in range(N_OT):
                                    nc.scalar.activation(
                                        h1_sb[:, ot * M_CHUNK + ms * MS:
                                              ot * M_CHUNK + (ms + 1) * MS],
                                        accs[ot][:, :],
                                        ID,
                                        bias=b1_sb[:, oc * N_OT + ot:
                                                   oc * N_OT + ot + 1],
                                        scale=1.0,
                                    )
                        else:
                            for ot in range(N_OT):
                                accs = [ps1.tile([128, MS], f32, tag="ps1",
                                                 name="ps1t")
                                        for _ in range(N_MS)]
                                for k in range(KT1):
                                    lhsT = w1_sb[:, k * O_CHUNK + ot * 128:
                                                 k * O_CHUNK + (ot + 1) * 128]
                                    for ms in range(N_MS):
                                        nc.tensor.matmul(
                                            accs[ms][:, :],
                                            lhsT,
                                            xt_sb[:, k * M_CHUNK + ms * MS:
                                                  k * M_CHUNK + (ms + 1) * MS],
                                            start=(k == 0),
                                            stop=(k == KT1 - 1),
                                        )
                                for ms in range(N_MS):
                                    nc.scalar.activation(
                                        h1_sb[:, ot * M_CHUNK + ms * MS:
                                              ot * M_CHUNK + (ms + 1) * MS],
                                        accs[ms][:, :],
                                        ID,
                                        bias=b1_sb[:, oc * N_OT + ot:
                                                   oc * N_OT + ot + 1],
                                        scale=1.0,
                                    )
                        h1_half.append(h1_sb)

                        # W2T slice [O_CHUNK, HIDDEN] -> [128, N_OT * HIDDEN].
                        # Emitted after GEMM1 so its DMA queues behind the
                        # critical-path x/W1 loads.
                        w2_sb = w2p.tile([128, N_OT * HIDDEN], f32r, tag="w2")
                        w2_args = (
                            w2_sb[:, :].rearrange("p (a n) -> p a n", n=HIDDEN),
                            w2T[o0:o0 + O_CHUNK, :].rearrange(
                                "(a p) n -> p a n", p=128),
                        )
                        if cold:
                            deferred_w2.append(w2_args)
                        else:
                            nc.sync.dma_start(*w2_args)
                        w2_half.append(w2_sb)

                    # GEMM2 for the o-pair: accumulate 8 k-steps in PSUM,
                    # then fold into h2_sb.
                    for ht in range(N_HT):
                        accs = [ps2.tile([128, MS], f32, tag="ps2", name="ps2t")
                                for _ in range(N_MS)]
                        for k in range(KT2):
                            half, ot = divmod(k, N_OT)
                            lhsT = w2_half[half][:, ot * HIDDEN + ht * 128:
                                                 ot * HIDDEN + (ht + 1) * 128]
                            for ms in range(N_MS):
                                nc.tensor.matmul(
                                    accs[ms][:, :],
                                    lhsT,
                                    h1_half[half][:, ot * M_CHUNK + ms * MS:
                                                  ot * M_CHUNK + (ms + 1) * MS],
                                    start=(k == 0),
                                    stop=(k == KT2 - 1),
                                )
                        for ms in range(N_MS):
                            dst = h2_sb[:, ht * M_CHUNK + ms * MS:
                                        ht * M_CHUNK + (ms + 1) * MS]
                            if pair == 0:
                                nc.scalar.activation(
                                    dst, accs[ms][:, :], ID,
                                    bias=b2_sb[:, ht:ht + 1], scale=1.0,
                                )
                            else:
                                nc.vector.tensor_add(dst, dst, accs[ms][:, :])
                        if pair == N_PAIR - 1:
                            # Stream each hout-row-block out as soon as its
                            # last fold lands — keeps the kernel tail short.
                            # One DMA per block: each dma_start pays ~2us of
                            # completion latency, so fewer, larger transfers
                            # win at the very end.
                            nc.sync.dma_start(
                                outT[ht * 128:(ht + 1) * 128,
                                     m0:m0 + M_CHUNK],
                                h2_sb[:, ht * M_CHUNK:(ht + 1) * M_CHUNK],
                            )

    nc.compile()
    _NC_CACHE = nc
    return nc


def _prep_core_inputs(x, W1, b1, W2, b2, i):
    return {
        "xT": np.ascontiguousarray(np.asarray(x[i], dtype=np.float32).T),
        "w1T": np.ascontiguousarray(np.asarray(W1[i], dtype=np.float32).T),
        "b1": np.ascontiguousarray(
            np.asarray(b1[i], dtype=np.float32).reshape(INTER // 128, 128).T),
        "w2T": np.ascontiguousarray(np.asarray(W2[i], dtype=np.float32).T),
        "b2": np.ascontiguousarray(
            np.asarray(b2[i], dtype=np.float32).reshape(HIDDEN // 128, 128).T),
    }


def kernel(x, W1, b1, W2, b2, _trace=False, _trace_kwargs=None):
    x = np.asarray(x, dtype=np.float32)
    orig_shape = x.shape
    xg = x.reshape(NUM_GEMMS, M, HIDDEN)

    nc = build_nc()
    in_maps = [_prep_core_inputs(xg, W1, b1, W2, b2, i) for i in range(NUM_GEMMS)]
    res = None
    for attempt in range(3):
        try:
            res = run_bass_kernel_spmd(
                nc, in_maps, list(range(NUM_GEMMS)),
                trace=_trace, **(_trace_kwargs or {}),
            )
            break
        except Exception:
            # transient NRT_EXEC_UNIT_UNRECOVERABLE has been observed on
            # rapid repeated runs; a short pause and retry recovers
            if attempt == 2:
                raise
            import time
            time.sleep(20)
    out = np.stack(
        [res.results[i]["outT"].T for i in range(NUM_GEMMS)], axis=0
    ).reshape(orig_shape).astype(np.float32)
    if _trace:
        return out, res
    return out



# revision 3
# speedup vs baseline: 1.0533x; 1.0533x over previous
"""Grouped-GEMM MoE expert MLP kernel for 8 Trainium2 NeuronCores.

Problem: x [8, 2048, 1024] f32, per-group W1 [8, 4096, 1024], b1 [8, 4096],
W2 [8, 1024, 4096], b2 [8, 1024] (torch Linear convention, y = x @ W.T + b):
  h1 = xg @ W1.T + b1        (per group)
  h2 = h1 @ W2.T + b2
Expert-parallel: core i owns group i entirely — no collectives.

Formulation is fully transposed so every DMA is contiguous and biases land on
the partition axis:
  h1T[o, m]   = matmul(lhsT=W1T[h,o] tiles, rhs=xT[h,m] tiles)  + b1[o]
  outT[ho, m] = matmul(lhsT=W2T[o,ho] tiles, rhs=h1T[o,m] tiles) + b2[ho]
(out = lhsT.T @ rhs contracts the partition axis of both operands.)
Host pre-transposes x/W1/W2 per shard and un-transposes the output.

Matmuls run in bfloat16 with fp32 PSUM accumulation and f32 biases.
Measured HW cadence: bf16 matmul [128k x 512f] = 215.8 ns back-to-back vs
fp32r's 226.7 ns (fp32r pays ~32 extra cycles per instruction for the 4-byte
weight load) — 2048 matmuls/core -> ~442 us PE floor.  End-to-end rel err
(vs f32 reference) ~3e-3, dominated by bf16 input rounding.

Per-core loop structure: 2 m-chunks of 1024 tokens; inside, 8 o-chunks of 512.
GEMM1 for an o-chunk feeds SBUF bf16 tiles h1T; GEMM2 accumulates PSUM over an
o-PAIR (1024, 8 k-steps) then folds into an f32 SBUF accumulator (first pair
via ScalarE copy+bias, later pairs via VectorE add) to keep PSUM pressure at
4+4 banks. Weights are streamed per m-chunk (2 x 16.8 MB bf16), x and out once.
"""
import sys

sys.path.insert(0, "/opt/trn_rl_repo")

import numpy as np

import concourse.bass as bass  # noqa: F401  (bass import initializes mybir deps)
import concourse.mybir as mybir
import concourse.tile as tile
from concourse import bacc
from concourse.bass_utils import run_bass_kernel_spmd

NUM_GEMMS = 8
HIDDEN = 1024
INTER = 4096
M = 2048  # tokens per group

M_CHUNK = 1024  # tokens per chunk (2 chunks)
MS = 512        # matmul moving free dim (one PSUM bank)
O_CHUNK = 512   # GEMM1 / weight-DMA granularity along INTER
O_PAIR = 1024   # GEMM2 PSUM accumulation span along INTER (8 k-steps)

f32 = mybir.dt.float32
bf16 = mybir.dt.bfloat16

N_MC = M // M_CHUNK              # 2
N_PAIR = INTER // O_PAIR         # 4
N_MS = M_CHUNK // MS             # 2
KT1 = HIDDEN // 128              # 8 k-tiles for GEMM1
KT2 = O_PAIR // 128              # 8 k-tiles per GEMM2 psum group
N_OT = O_CHUNK // 128            # 4 o-tiles per o-chunk
N_HT = HIDDEN // 128             # 8 hout-tiles
N_WARM = 26                      # bf16 warmup matmuls (HAM clock ramp)

_NC_CACHE = None


def build_nc():
    """Build + compile the single-core program (same on all 8 cores)."""
    global _NC_CACHE
    if _NC_CACHE is not None:
        return _NC_CACHE

    nc = bacc.Bacc("TRN2", target_bir_lowering=False, debug=False, num_devices=8)
    xT = nc.dram_tensor("xT", [HIDDEN, M], bf16, kind="ExternalInput").ap()
    w1T = nc.dram_tensor("w1T", [HIDDEN, INTER], bf16, kind="ExternalInput").ap()
    b1 = nc.dram_tensor("b1", [128, INTER // 128], f32, kind="ExternalInput").ap()
    w2T = nc.dram_tensor("w2T", [INTER, HIDDEN], bf16, kind="ExternalInput").ap()
    b2 = nc.dram_tensor("b2", [128, HIDDEN // 128], f32, kind="ExternalInput").ap()
    outT = nc.dram_tensor("outT", [HIDDEN, M], f32, kind="ExternalOutput").ap()

    ID = mybir.ActivationFunctionType.Identity

    with tile.TileContext(nc) as tc:
        with (
            tc.tile_pool(name="cst", bufs=1) as cst,
            tc.tile_pool(name="xp", bufs=1) as xp,
            tc.tile_pool(name="hp", bufs=1) as hp,
            tc.tile_pool(name="w1p", bufs=3) as w1p,
            tc.tile_pool(name="w2p", bufs=2) as w2p,
            tc.tile_pool(name="h1p", bufs=2) as h1p,
            tc.tile_pool(name="ps1", bufs=4, space="PSUM") as ps1,
            tc.tile_pool(name="ps2", bufs=4, space="PSUM") as ps2,
        ):
            # PE warmup while the first DMAs fill: releases the HAM clock
            # throttle (4/8 -> 8/8, needs ~3.6us of gapless PE activity)
            # before the real matmuls arrive.  Warmup operands are DVE-memset
            # bf16 tiles (no DMA dependency), so the chain starts as soon as
            # the Tensor queue clears its preamble (~7.2us) and runs gapless
            # at 216-432ns per matmul until the first weight/x tiles land.
            wwarm = cst.tile([128, 128], bf16)
            xwarm = cst.tile([128, 512], bf16)
            nc.vector.memset(wwarm[:, :], 0.01)
            nc.vector.memset(xwarm[:, :], 0.5)
            for _ in range(N_WARM):
                ps_junk = ps1.tile([128, MS], f32, tag="ps1", name="ps1t")
                nc.tensor.matmul(ps_junk[:, :], wwarm[:, :], xwarm[:, :],
                                 start=True, stop=True)

            b1_sb = cst.tile([128, INTER // 128], f32)
            b2_sb = cst.tile([128, HIDDEN // 128], f32)

            for mc in range(N_MC):
                m0 = mc * M_CHUNK
                # x chunk: [HIDDEN, M_CHUNK] -> [128, KT1 * M_CHUNK].
                # Split per k-tile so the first GEMM1 matmuls can start as
                # soon as k-tile 0 lands (subtile deps) instead of after the
                # whole 2.1 MB chunk.
                xt_sb = xp.tile([128, KT1 * M_CHUNK], bf16, tag="xt")
                if mc != 0:
                    # Non-first chunk: one bulk prefetch queued behind
                    # current work.
                    nc.sync.dma_start(
                        xt_sb[:, :].rearrange("p (a m) -> p a m", m=M_CHUNK),
                        xT[:, m0:m0 + M_CHUNK].rearrange(
                            "(a p) m -> p a m", p=128),
                    )
                # output accumulator: [HIDDEN, M_CHUNK] -> [128, N_HT * M_CHUNK]
                h2_sb = hp.tile([128, N_HT * M_CHUNK], f32, tag="h2")

                for pair in range(N_PAIR):
                    h1_half = []
                    w2_half = []
                    deferred_w2 = []
                    for half in range(2):
                        oc = pair * 2 + half
                        o0 = oc * O_CHUNK
                        cold = mc == 0 and pair == 0 and half == 0
                        # W1T slice [HIDDEN, O_CHUNK] -> [128, KT1 * O_CHUNK]
                        w1_sb = w1p.tile([128, KT1 * O_CHUNK], bf16, tag="w1")
                        if cold:
                            # Cold fill, ordered to match the ms-outer
                            # consumption order of the first GEMM1 pass and
                            # spread across FOUR queues so the ~600ns
                            # per-dma_start issue cost parallelizes: sync
                            # carries W1 k-tiles, scalar/vector/gpsimd carry
                            # the x halves and biases.
                            def xt_half(k, ms):
                                return (
                                    xt_sb[:, k * M_CHUNK + ms * MS:
                                          k * M_CHUNK + (ms + 1) * MS],
                                    xT[k * 128:(k + 1) * 128,
                                       m0 + ms * MS:m0 + (ms + 1) * MS],
                                )
                            nc.sync.dma_start(
                                w1_sb[:, 0:O_CHUNK],
                                w1T[0:128, o0:o0 + O_CHUNK],
                            )
                            nc.scalar.dma_start(*xt_half(0, 0))
                            nc.gpsimd.dma_start(*xt_half(0, 1))
                            nc.scalar.dma_start(b1_sb[:, :], b1[:, :])
                            nc.scalar.dma_start(b2_sb[:, :], b2[:, :])
                            for k in range(1, KT1):
                                nc.sync.dma_start(
                                    w1_sb[:, k * O_CHUNK:(k + 1) * O_CHUNK],
                                    w1T[k * 128:(k + 1) * 128, o0:o0 + O_CHUNK],
                                )
                                nc.scalar.dma_start(*xt_half(k, 0))
                                nc.gpsimd.dma_start(*xt_half(k, 1))
                        else:
                            nc.sync.dma_start(
                                w1_sb[:, :].rearrange("p (a o) -> p a o",
                                                      o=O_CHUNK),
                                w1T[:, o0:o0 + O_CHUNK].rearrange(
                                    "(a p) o -> p a o", p=128),
                            )
                        # The cold half's W2 slice queues here, AFTER this
                        # half's W1 — W1(oc1) is needed ~15us before
                        # W2(oc0), and the sync queue delivers in FIFO
                        # order.
                        for args in deferred_w2:
                            nc.sync.dma_start(*args)
                        deferred_w2 = []

                        # GEMM1: h1T[o0:o0+512, m-chunk] in bf16
                        h1_sb = h1p.tile([128, N_OT * M_CHUNK], bf16, tag="h1")
                        if cold:
                            # k-outer order: consume k-tiles as they arrive.
                            # ms outer keeps live PSUM groups at N_OT = 4.
                            for ms in range(N_MS):
                                accs = [ps1.tile([128, MS], f32, tag="ps1",
                                                 name="ps1t")
                                        for _ in range(N_OT)]
                                for k in range(KT1):
                                    for ot in range(N_OT):
                                        nc.tensor.matmul(
                                            accs[ot][:, :],
                                            w1_sb[:, k * O_CHUNK + ot * 128:
                                                  k * O_CHUNK + (ot + 1) * 128],
                                            xt_sb[:, k * M_CHUNK + ms * MS:
                                                  k * M_CHUNK + (ms + 1) * MS],
                                            start=(k == 0),
                                            stop=(k == KT1 - 1),
                                        )
                                for ot in range(N_OT):
                                    nc.scalar.activation(
                                        h1_sb[:, ot * M_CHUNK + ms * MS:
                                              ot * M_CHUNK + (ms + 1) * MS],
                                        accs[ot][:, :],
                                        ID,
                                        bias=b1_sb[:, oc * N_OT + ot:
                                                   oc * N_OT + ot + 1],
                                        scale=1.0,
                                    )
                        else:
                            for ot in range(N_OT):
                                accs = [ps1.tile([128, MS], f32, tag="ps1",
                                                 name="ps1t")
                                        for _ in range(N_MS)]
                                for k in range(KT1):
                                    lhsT = w1_sb[:, k * O_CHUNK + ot * 128:
                                                 k * O_CHUNK + (ot + 1) * 128]
                                    for ms in range(N_MS):
                                        nc.tensor.matmul(
                                            accs[ms][:, :],
                                            lhsT,
                                            xt_sb[:, k * M_CHUNK + ms * MS:
                                                  k * M_CHUNK + (ms + 1) * MS],
                                            start=(k == 0),
                                            stop=(k == KT1 - 1),
                                        )
                                for ms in range(N_MS):
                                    nc.scalar.activation(
                                        h1_sb[:, ot * M_CHUNK + ms * MS:
                                              ot * M_CHUNK + (ms + 1) * MS],
                                        accs[ms][:, :],
                                        ID,
                                        bias=b1_sb[:, oc * N_OT + ot:
                                                   oc * N_OT + ot + 1],
                                        scale=1.0,
                                    )
                        h1_half.append(h1_sb)

                        # W2T slice [O_CHUNK, HIDDEN] -> [128, N_OT * HIDDEN].
                        # Emitted after GEMM1 so its DMA queues behind the
                        # critical-path x/W1 loads.
                        w2_sb = w2p.tile([128, N_OT * HIDDEN], bf16, tag="w2")
                        w2_args = (
                            w2_sb[:, :].rearrange("p (a n) -> p a n", n=HIDDEN),
                            w2T[o0:o0 + O_CHUNK, :].rearrange(
                                "(a p) n -> p a n", p=128),
                        )
                        if cold:
                            deferred_w2.append(w2_args)
                        else:
                            nc.sync.dma_start(*w2_args)
                        w2_half.append(w2_sb)

                    # GEMM2 for the o-pair: accumulate 8 k-steps in PSUM,
                    # then fold into h2_sb.
                    for ht in range(N_HT):
                        accs = [ps2.tile([128, MS], f32, tag="ps2", name="ps2t")
                                for _ in range(N_MS)]
                        for k in range(KT2):
                            half, ot = divmod(k, N_OT)
                            lhsT = w2_half[half][:, ot * HIDDEN + ht * 128:
                                                 ot * HIDDEN + (ht + 1) * 128]
                            for ms in range(N_MS):
                                nc.tensor.matmul(
                                    accs[ms][:, :],
                                    lhsT,
                                    h1_half[half][:, ot * M_CHUNK + ms * MS:
                                                  ot * M_CHUNK + (ms + 1) * MS],
                                    start=(k == 0),
                                    stop=(k == KT2 - 1),
                                )
                        for ms in range(N_MS):
                            dst = h2_sb[:, ht * M_CHUNK + ms * MS:
                                        ht * M_CHUNK + (ms + 1) * MS]
                            if pair == 0:
                                nc.scalar.activation(
                                    dst, accs[ms][:, :], ID,
                                    bias=b2_sb[:, ht:ht + 1], scale=1.0,
                                )
                            else:
                                nc.vector.tensor_add(dst, dst, accs[ms][:, :])
                        if pair == N_PAIR - 1:
                            # Stream each hout-row-block out as soon as its
                            # last fold lands — keeps the kernel tail short.
                            # The very last block goes out as two half-DMAs
                            # on different queues so the final transfer is
                            # 256KB instead of 512KB.
                            last = mc == N_MC - 1 and ht == N_HT - 1
                            if last:
                                nc.sync.dma_start(
                                    outT[ht * 128:(ht + 1) * 128,
                                         m0:m0 + MS],
                                    h2_sb[:, ht * M_CHUNK:
                                          ht * M_CHUNK + MS],
                                )
                                nc.scalar.dma_start(
                                    outT[ht * 128:(ht + 1) * 128,
                                         m0 + MS:m0 + M_CHUNK],
                                    h2_sb[:, ht * M_CHUNK + MS:
                                          ht * M_CHUNK + M_CHUNK],
                                )
                            else:
                                nc.sync.dma_start(
                                    outT[ht * 128:(ht + 1) * 128,
                                         m0:m0 + M_CHUNK],
                                    h2_sb[:, ht * M_CHUNK:(ht + 1) * M_CHUNK],
                                )

    nc.compile()
    _NC_CACHE = nc
    return nc


def _prep_core_inputs(x, W1, b1, W2, b2, i):
    bf = np.dtype("bfloat16") if hasattr(np, "bfloat16") else None
    import ml_dtypes
    bf = ml_dtypes.bfloat16
    return {
        "xT": np.ascontiguousarray(
            np.asarray(x[i], dtype=np.float32).T).astype(bf),
        "w1T": np.ascontiguousarray(
            np.asarray(W1[i], dtype=np.float32).T).astype(bf),
        "b1": np.ascontiguousarray(
            np.asarray(b1[i], dtype=np.float32).reshape(INTER // 128, 128).T),
        "w2T": np.ascontiguousarray(
            np.asarray(W2[i], dtype=np.float32).T).astype(bf),
        "b2": np.ascontiguousarray(
            np.asarray(b2[i], dtype=np.float32).reshape(HIDDEN // 128, 128).T),
    }


def kernel(x, W1, b1, W2, b2, _trace=False, _trace_kwargs=None):
    x = np.asarray(x, dtype=np.float32)
    orig_shape = x.shape
    xg = x.reshape(NUM_GEMMS, M, HIDDEN)

    nc = build_nc()
    in_maps = [_prep_core_inputs(xg, W1, b1, W2, b2, i) for i in range(NUM_GEMMS)]
    res = None
    for attempt in range(3):
        try:
            res = run_bass_kernel_spmd(
                nc, in_maps, list(range(NUM_GEMMS)),
                trace=_trace, **(_trace_kwargs or {}),
            )
            break
        except Exception:
            # transient NRT_EXEC_UNIT_UNRECOVERABLE has been observed on
            # rapid repeated runs; a short pause and retry recovers
            if attempt == 2:
                raise
            import time
            time.sleep(20)
    out = np.stack(
        [res.results[i]["outT"].T for i in range(NUM_GEMMS)], axis=0
    ).reshape(orig_shape).astype(np.float32)
    if _trace:
        return out, res
    return out


# revision 7
# speedup vs baseline: 1.0604x; 1.0067x over previous
"""Grouped-GEMM MoE expert MLP kernel for 8 Trainium2 NeuronCores.

Problem: x [8, 2048, 1024] f32, per-group W1 [8, 4096, 1024], b1 [8, 4096],
W2 [8, 1024, 4096], b2 [8, 1024] (torch Linear convention, y = x @ W.T + b):
  h1 = xg @ W1.T + b1        (per group)
  h2 = h1 @ W2.T + b2
Expert-parallel: core i owns group i entirely — no collectives.

Formulation is fully transposed so every DMA is contiguous and biases land on
the partition axis:
  h1T[o, m]   = matmul(lhsT=W1T[h,o] tiles, rhs=xT[h,m] tiles)  + b1[o]
  outT[ho, m] = matmul(lhsT=W2T[o,ho] tiles, rhs=h1T[o,m] tiles) + b2[ho]
(out = lhsT.T @ rhs contracts the partition axis of both operands.)
Host pre-transposes x/W1/W2 per shard and un-transposes the output.

Matmuls run in bfloat16 with fp32 PSUM accumulation and f32 biases.
Measured HW cadence: bf16 matmul [128k x 512f] = 215.8 ns back-to-back vs
fp32r's 226.7 ns (fp32r pays ~32 extra cycles per instruction for the 4-byte
weight load) — 2048 matmuls/core -> ~442 us PE floor.  End-to-end rel err
(vs f32 reference) ~3e-3, dominated by bf16 input rounding.

Per-core loop structure: 2 m-chunks of 1024 tokens; inside, 8 o-chunks of 512.
GEMM1 for an o-chunk feeds SBUF bf16 tiles h1T; GEMM2 accumulates PSUM over an
o-PAIR (1024, 8 k-steps) then folds into an f32 SBUF accumulator (first pair
via ScalarE copy+bias, later pairs via VectorE add) to keep PSUM pressure at
4+4 banks. Weights are streamed per m-chunk (2 x 16.8 MB bf16), x and out once.
"""
import sys

sys.path.insert(0, "/opt/trn_rl_repo")

import numpy as np

import concourse.bass as bass  # noqa: F401  (bass import initializes mybir deps)
import concourse.mybir as mybir
import concourse.tile as tile
from concourse import bacc
from concourse.bass_utils import run_bass_kernel_spmd

NUM_GEMMS = 8
HIDDEN = 1024
INTER = 4096
M = 2048  # tokens per group

M_CHUNK = 1024  # tokens per chunk (2 chunks)
MS = 512        # matmul moving free dim (one PSUM bank)
O_CHUNK = 512   # GEMM1 / weight-DMA granularity along INTER
O_PAIR = 1024   # GEMM2 PSUM accumulation span along INTER (8 k-steps)

f32 = mybir.dt.float32
bf16 = mybir.dt.bfloat16

N_MC = M // M_CHUNK              # 2
N_PAIR = INTER // O_PAIR         # 4
N_MS = M_CHUNK // MS             # 2
KT1 = HIDDEN // 128              # 8 k-tiles for GEMM1
KT2 = O_PAIR // 128              # 8 k-tiles per GEMM2 psum group
N_OT = O_CHUNK // 128            # 4 o-tiles per o-chunk
N_HT = HIDDEN // 128             # 8 hout-tiles
N_WARM = 6                       # bf16 warmup matmuls (HAM clock ramp)

_NC_CACHE = None


def build_nc():
    """Build + compile the single-core program (same on all 8 cores)."""
    global _NC_CACHE
    if _NC_CACHE is not None:
        return _NC_CACHE

    nc = bacc.Bacc("TRN2", target_bir_lowering=False, debug=False, num_devices=8)
    xT = nc.dram_tensor("xT", [HIDDEN, M], bf16, kind="ExternalInput").ap()
    w1T = nc.dram_tensor("w1T", [HIDDEN, INTER], bf16, kind="ExternalInput").ap()
    b1 = nc.dram_tensor("b1", [128, INTER // 128], f32, kind="ExternalInput").ap()
    w2T = nc.dram_tensor("w2T", [INTER, HIDDEN], bf16, kind="ExternalInput").ap()
    b2 = nc.dram_tensor("b2", [128, HIDDEN // 128], f32, kind="ExternalInput").ap()
    outT = nc.dram_tensor("outT", [HIDDEN, M], f32, kind="ExternalOutput").ap()

    ID = mybir.ActivationFunctionType.Identity

    with tile.TileContext(nc) as tc:
        with (
            tc.tile_pool(name="cst", bufs=1) as cst,
            tc.tile_pool(name="xp", bufs=1) as xp,
            tc.tile_pool(name="hp", bufs=1) as hp,
            tc.tile_pool(name="w1p", bufs=3) as w1p,
            tc.tile_pool(name="w2p", bufs=2) as w2p,
            tc.tile_pool(name="h1p", bufs=2) as h1p,
            tc.tile_pool(name="ps1", bufs=4, space="PSUM") as ps1,
            tc.tile_pool(name="ps2", bufs=4, space="PSUM") as ps2,
        ):
            # PE warmup while the first DMAs fill: starts the HAM clock
            # throttle release window (4/8 -> 8/8 after ~5.9us of gapless PE
            # activity) as early as possible.  First a const-operand fp32
            # matmul (available the moment the Tensor queue clears its
            # preamble ~7.3us, runs ~2.2us at the throttled clock), then a
            # few bf16 matmuls on DVE-memset tiles to bridge until the first
            # weight/x tiles land (~11us).  Real matmuls take over at half
            # clock for ~2us until the release at ~13.2us.
            ps_junk = ps1.tile([128, MS], f32, tag="ps1", name="ps1t")
            cwarm = nc.const_aps.scalar_like(1.0, ps_junk[:, :])
            cbr = cwarm.broadcast_to([128, MS])
            nc.tensor.matmul(ps_junk[:1, :], cwarm, cbr, start=True, stop=True)
            wwarm = cst.tile([128, 128], bf16)
            xwarm = cst.tile([128, 512], bf16)
            nc.vector.memset(wwarm[:, :], 0.01)
            nc.vector.memset(xwarm[:, :], 0.5)
            for _ in range(N_WARM):
                ps_junk = ps1.tile([128, MS], f32, tag="ps1", name="ps1t")
                nc.tensor.matmul(ps_junk[:, :], wwarm[:, :], xwarm[:, :],
                                 start=True, stop=True)

            b1_sb = cst.tile([128, INTER // 128], f32)
            b2_sb = cst.tile([128, HIDDEN // 128], f32)

            for mc in range(N_MC):
                m0 = mc * M_CHUNK
                # x chunk: [HIDDEN, M_CHUNK] -> [128, KT1 * M_CHUNK].
                # Split per k-tile so the first GEMM1 matmuls can start as
                # soon as k-tile 0 lands (subtile deps) instead of after the
                # whole 2.1 MB chunk.
                xt_sb = xp.tile([128, KT1 * M_CHUNK], bf16, tag="xt")
                if mc != 0:
                    # Non-first chunk: one bulk prefetch queued behind
                    # current work.
                    nc.sync.dma_start(
                        xt_sb[:, :].rearrange("p (a m) -> p a m", m=M_CHUNK),
                        xT[:, m0:m0 + M_CHUNK].rearrange(
                            "(a p) m -> p a m", p=128),
                    )
                # output accumulator: [HIDDEN, M_CHUNK] -> [128, N_HT * M_CHUNK]
                h2_sb = hp.tile([128, N_HT * M_CHUNK], f32, tag="h2")

                for pair in range(N_PAIR):
                    h1_half = []
                    w2_half = []
                    deferred_w2 = []
                    for half in range(2):
                        oc = pair * 2 + half
                        o0 = oc * O_CHUNK
                        cold = mc == 0 and pair == 0 and half == 0
                        # W1T slice [HIDDEN, O_CHUNK] -> [128, KT1 * O_CHUNK]
                        w1_sb = w1p.tile([128, KT1 * O_CHUNK], bf16, tag="w1")
                        if cold:
                            # Cold fill, spread across THREE dma-capable
                            # queues (sync/scalar/gpsimd) so the ~600ns
                            # per-dma_start issue cost and the per-queue
                            # serialized transfers parallelize.  The ms0
                            # critical set (x k-tiles ms0 + all W1 k-tiles,
                            # 2MB) is balanced so it completes by ~11-12us;
                            # the ms1 x half trails on gpsimd.
                            def w1_args(k):
                                return (
                                    w1_sb[:, k * O_CHUNK:(k + 1) * O_CHUNK],
                                    w1T[k * 128:(k + 1) * 128, o0:o0 + O_CHUNK],
                                )

                            def xt_half(k, ms):
                                return (
                                    xt_sb[:, k * M_CHUNK + ms * MS:
                                          k * M_CHUNK + (ms + 1) * MS],
                                    xT[k * 128:(k + 1) * 128,
                                       m0 + ms * MS:m0 + (ms + 1) * MS],
                                )
                            # sync: W1 k0-k3, then x k4-7 ms0
                            # scalar: x k0-3 ms0, then biases
                            # gpsimd: W1 k4-7, then x ms1
                            nc.sync.dma_start(*w1_args(0))
                            nc.scalar.dma_start(*xt_half(0, 0))
                            nc.gpsimd.dma_start(*w1_args(4))
                            for k in range(1, 4):
                                nc.sync.dma_start(*w1_args(k))
                                nc.scalar.dma_start(*xt_half(k, 0))
                                nc.gpsimd.dma_start(*w1_args(k + 4))
                            for k in range(4, KT1):
                                nc.sync.dma_start(*xt_half(k, 0))
                            nc.scalar.dma_start(b1_sb[:, :], b1[:, :])
                            nc.scalar.dma_start(b2_sb[:, :], b2[:, :])
                            for k in range(KT1):
                                nc.gpsimd.dma_start(*xt_half(k, 1))
                        else:
                            nc.sync.dma_start(
                                w1_sb[:, :].rearrange("p (a o) -> p a o",
                                                      o=O_CHUNK),
                                w1T[:, o0:o0 + O_CHUNK].rearrange(
                                    "(a p) o -> p a o", p=128),
                            )
                        # The cold half's W2 slice queues here, AFTER this
                        # half's W1 — W1(oc1) is needed ~15us before
                        # W2(oc0), and the sync queue delivers in FIFO
                        # order.
                        for args in deferred_w2:
                            nc.sync.dma_start(*args)
                        deferred_w2 = []

                        # GEMM1: h1T[o0:o0+512, m-chunk] in bf16
                        h1_sb = h1p.tile([128, N_OT * M_CHUNK], bf16, tag="h1")
                        if cold:
                            # k-outer order: consume k-tiles as they arrive.
                            # ms outer keeps live PSUM groups at N_OT = 4.
                            for ms in range(N_MS):
                                accs = [ps1.tile([128, MS], f32, tag="ps1",
                                                 name="ps1t")
                                        for _ in range(N_OT)]
                                for k in range(KT1):
                                    for ot in range(N_OT):
                                        nc.tensor.matmul(
                                            accs[ot][:, :],
                                            w1_sb[:, k * O_CHUNK + ot * 128:
                                                  k * O_CHUNK + (ot + 1) * 128],
                                            xt_sb[:, k * M_CHUNK + ms * MS:
                                                  k * M_CHUNK + (ms + 1) * MS],
                                            start=(k == 0),
                                            stop=(k == KT1 - 1),
                                        )
                                for ot in range(N_OT):
                                    nc.scalar.activation(
                                        h1_sb[:, ot * M_CHUNK + ms * MS:
                                              ot * M_CHUNK + (ms + 1) * MS],
                                        accs[ot][:, :],
                                        ID,
                                        bias=b1_sb[:, oc * N_OT + ot:
                                                   oc * N_OT + ot + 1],
                                        scale=1.0,
                                    )
                        else:
                            for ot in range(N_OT):
                                accs = [ps1.tile([128, MS], f32, tag="ps1",
                                                 name="ps1t")
                                        for _ in range(N_MS)]
                                for k in range(KT1):
                                    lhsT = w1_sb[:, k * O_CHUNK + ot * 128:
                                                 k * O_CHUNK + (ot + 1) * 128]
                                    for ms in range(N_MS):
                                        nc.tensor.matmul(
                                            accs[ms][:, :],
                                            lhsT,
                                            xt_sb[:, k * M_CHUNK + ms * MS:
                                                  k * M_CHUNK + (ms + 1) * MS],
                                            start=(k == 0),
                                            stop=(k == KT1 - 1),
                                        )
                                for ms in range(N_MS):
                                    nc.scalar.activation(
                                        h1_sb[:, ot * M_CHUNK + ms * MS:
                                              ot * M_CHUNK + (ms + 1) * MS],
                                        accs[ms][:, :],
                                        ID,
                                        bias=b1_sb[:, oc * N_OT + ot:
                                                   oc * N_OT + ot + 1],
                                        scale=1.0,
                                    )
                        h1_half.append(h1_sb)

                        # W2T slice [O_CHUNK, HIDDEN] -> [128, N_OT * HIDDEN].
                        # Emitted after GEMM1 so its DMA queues behind the
                        # critical-path x/W1 loads.
                        w2_sb = w2p.tile([128, N_OT * HIDDEN], bf16, tag="w2")
                        w2_args = (
                            w2_sb[:, :].rearrange("p (a n) -> p a n", n=HIDDEN),
                            w2T[o0:o0 + O_CHUNK, :].rearrange(
                                "(a p) n -> p a n", p=128),
                        )
                        if cold:
                            deferred_w2.append(w2_args)
                        else:
                            nc.sync.dma_start(*w2_args)
                        w2_half.append(w2_sb)

                    # GEMM2 for the o-pair: accumulate 8 k-steps in PSUM,
                    # then fold into h2_sb.
                    # On the final pair of the final m-chunk the k-loop runs
                    # ms-split (all ms0 steps, fold, half-DMA, then ms1) so
                    # the last output transfers overlap the remaining
                    # matmuls and the tail after the last matmul is just one
                    # 256KB half-DMA + fold.
                    tail_pair = pair == N_PAIR - 1 and mc == N_MC - 1
                    for ht in range(N_HT):
                        accs = [ps2.tile([128, MS], f32, tag="ps2", name="ps2t")
                                for _ in range(N_MS)]
                        ms_groups = ([[0, 1]] if not tail_pair
                                     else [[0], [1]])
                        for ms_grp in ms_groups:
                            for k in range(KT2):
                                half, ot = divmod(k, N_OT)
                                lhsT = w2_half[half][:, ot * HIDDEN + ht * 128:
                                                     ot * HIDDEN + (ht + 1) * 128]
                                for ms in ms_grp:
                                    nc.tensor.matmul(
                                        accs[ms][:, :],
                                        lhsT,
                                        h1_half[half][:, ot * M_CHUNK + ms * MS:
                                                      ot * M_CHUNK + (ms + 1) * MS],
                                        start=(k == 0),
                                        stop=(k == KT2 - 1),
                                    )
                            for ms in ms_grp:
                                dst = h2_sb[:, ht * M_CHUNK + ms * MS:
                                            ht * M_CHUNK + (ms + 1) * MS]
                                if pair == 0:
                                    nc.scalar.activation(
                                        dst, accs[ms][:, :], ID,
                                        bias=b2_sb[:, ht:ht + 1], scale=1.0,
                                    )
                                else:
                                    nc.vector.tensor_add(dst, dst,
                                                         accs[ms][:, :])
                                if tail_pair:
                                    # per-half output DMA, alternating queues
                                    q = nc.sync if ms == 0 else nc.scalar
                                    q.dma_start(
                                        outT[ht * 128:(ht + 1) * 128,
                                             m0 + ms * MS:m0 + (ms + 1) * MS],
                                        h2_sb[:, ht * M_CHUNK + ms * MS:
                                              ht * M_CHUNK + (ms + 1) * MS],
                                    )
                        if pair == N_PAIR - 1 and not tail_pair:
                            # Stream each hout-row-block out as soon as its
                            # last fold lands — keeps the kernel tail short.
                            nc.sync.dma_start(
                                outT[ht * 128:(ht + 1) * 128,
                                     m0:m0 + M_CHUNK],
                                h2_sb[:, ht * M_CHUNK:(ht + 1) * M_CHUNK],
                            )

    nc.compile()
    _NC_CACHE = nc
    return nc


def _prep_core_inputs(x, W1, b1, W2, b2, i):
    bf = np.dtype("bfloat16") if hasattr(np, "bfloat16") else None
    import ml_dtypes
    bf = ml_dtypes.bfloat16
    return {
        "xT": np.ascontiguousarray(
            np.asarray(x[i], dtype=np.float32).T).astype(bf),
        "w1T": np.ascontiguousarray(
            np.asarray(W1[i], dtype=np.float32).T).astype(bf),
        "b1": np.ascontiguousarray(
            np.asarray(b1[i], dtype=np.float32).reshape(INTER // 128, 128).T),
        "w2T": np.ascontiguousarray(
            np.asarray(W2[i], dtype=np.float32).T).astype(bf),
        "b2": np.ascontiguousarray(
            np.asarray(b2[i], dtype=np.float32).reshape(HIDDEN // 128, 128).T),
    }


def kernel(x, W1, b1, W2, b2, _trace=False, _trace_kwargs=None):
    x = np.asarray(x, dtype=np.float32)
    orig_shape = x.shape
    xg = x.reshape(NUM_GEMMS, M, HIDDEN)

    nc = build_nc()
    in_maps = [_prep_core_inputs(xg, W1, b1, W2, b2, i) for i in range(NUM_GEMMS)]
    res = None
    for attempt in range(3):
        try:
            res = run_bass_kernel_spmd(
                nc, in_maps, list(range(NUM_GEMMS)),
                trace=_trace, **(_trace_kwargs or {}),
            )
            break
        except Exception:
            # transient NRT_EXEC_UNIT_UNRECOVERABLE has been observed on
            # rapid repeated runs; a short pause and retry recovers
            if attempt == 2:
                raise
            import time
            time.sleep(20)
    out = np.stack(
        [res.results[i]["outT"].T for i in range(NUM_GEMMS)], axis=0
    ).reshape(orig_shape).astype(np.float32)
    if _trace:
        return out, res
    return out


# revision 9
# speedup vs baseline: 1.0655x; 1.0048x over previous
"""Grouped-GEMM MoE expert MLP kernel for 8 Trainium2 NeuronCores.

Problem: x [8, 2048, 1024] f32, per-group W1 [8, 4096, 1024], b1 [8, 4096],
W2 [8, 1024, 4096], b2 [8, 1024] (torch Linear convention, y = x @ W.T + b):
  h1 = xg @ W1.T + b1        (per group)
  h2 = h1 @ W2.T + b2
Expert-parallel: core i owns group i entirely — no collectives.

Formulation is fully transposed so every DMA is contiguous and biases land on
the partition axis:
  h1T[o, m]   = matmul(lhsT=W1T[h,o] tiles, rhs=xT[h,m] tiles)  + b1[o]
  outT[ho, m] = matmul(lhsT=W2T[o,ho] tiles, rhs=h1T[o,m] tiles) + b2[ho]
(out = lhsT.T @ rhs contracts the partition axis of both operands.)
Host pre-transposes x/W1/W2 per shard and un-transposes the output.

Matmuls run in bfloat16 with fp32 PSUM accumulation and f32 biases.
Measured HW cadence: bf16 matmul [128k x 512f] = 215.8 ns back-to-back vs
fp32r's 226.7 ns (fp32r pays ~32 extra cycles per instruction for the 4-byte
weight load) — 2048 matmuls/core -> ~442 us PE floor.  End-to-end rel err
(vs f32 reference) ~3e-3, dominated by bf16 input rounding.

Per-core loop structure: 2 m-chunks of 1024 tokens; inside, 8 o-chunks of 512.
GEMM1 for an o-chunk feeds SBUF bf16 tiles h1T; GEMM2 accumulates PSUM over an
o-PAIR (1024, 8 k-steps) then folds into an f32 SBUF accumulator (first pair
via ScalarE copy+bias, later pairs via VectorE add) to keep PSUM pressure at
4+4 banks. Weights are streamed per m-chunk (2 x 16.8 MB bf16), x and out once.
"""
import sys

sys.path.insert(0, "/opt/trn_rl_repo")

import numpy as np

import concourse.bass as bass  # noqa: F401  (bass import initializes mybir deps)
import concourse.mybir as mybir
import concourse.tile as tile
from concourse import bacc
from concourse.bass_utils import run_bass_kernel_spmd

NUM_GEMMS = 8
HIDDEN = 1024
INTER = 4096
M = 2048  # tokens per group

M_CHUNK = 1024  # tokens per chunk (2 chunks)
MS = 512        # matmul moving free dim (one PSUM bank)
O_CHUNK = 512   # GEMM1 / weight-DMA granularity along INTER
O_PAIR = 1024   # GEMM2 PSUM accumulation span along INTER (8 k-steps)

f32 = mybir.dt.float32
bf16 = mybir.dt.bfloat16

N_MC = M // M_CHUNK              # 2
N_PAIR = INTER // O_PAIR         # 4
N_MS = M_CHUNK // MS             # 2
KT1 = HIDDEN // 128              # 8 k-tiles for GEMM1
KT2 = O_PAIR // 128              # 8 k-tiles per GEMM2 psum group
N_OT = O_CHUNK // 128            # 4 o-tiles per o-chunk
N_HT = HIDDEN // 128             # 8 hout-tiles
N_WARM = 6                       # bf16 warmup matmuls (HAM clock ramp)

_NC_CACHE = None


def build_nc():
    """Build + compile the single-core program (same on all 8 cores)."""
    global _NC_CACHE
    if _NC_CACHE is not None:
        return _NC_CACHE

    nc = bacc.Bacc("TRN2", target_bir_lowering=False, debug=False, num_devices=8)
    xT = nc.dram_tensor("xT", [HIDDEN, M], bf16, kind="ExternalInput").ap()
    w1T = nc.dram_tensor("w1T", [HIDDEN, INTER], bf16, kind="ExternalInput").ap()
    b1 = nc.dram_tensor("b1", [128, INTER // 128], f32, kind="ExternalInput").ap()
    w2T = nc.dram_tensor("w2T", [INTER, HIDDEN], bf16, kind="ExternalInput").ap()
    b2 = nc.dram_tensor("b2", [128, HIDDEN // 128], f32, kind="ExternalInput").ap()
    outT = nc.dram_tensor("outT", [HIDDEN, M], f32, kind="ExternalOutput").ap()

    ID = mybir.ActivationFunctionType.Identity

    with tile.TileContext(nc) as tc:
        with (
            tc.tile_pool(name="cst", bufs=1) as cst,
            tc.tile_pool(name="xp", bufs=1) as xp,
            tc.tile_pool(name="hp", bufs=1) as hp,
            tc.tile_pool(name="w1p", bufs=3) as w1p,
            tc.tile_pool(name="w2p", bufs=2) as w2p,
            tc.tile_pool(name="h1p", bufs=2) as h1p,
            tc.tile_pool(name="ps1", bufs=4, space="PSUM") as ps1,
            tc.tile_pool(name="ps2", bufs=4, space="PSUM") as ps2,
        ):
            # PE warmup while the first DMAs fill: starts the HAM clock
            # throttle release window (4/8 -> 8/8 after ~5.9us of gapless PE
            # activity) as early as possible.  First a const-operand fp32
            # matmul (available the moment the Tensor queue clears its
            # preamble ~7.3us, runs ~2.2us at the throttled clock), then a
            # few bf16 matmuls on DVE-memset tiles to bridge until the first
            # weight/x tiles land (~11us).  Real matmuls take over at half
            # clock for ~2us until the release at ~13.2us.
            ps_junk = ps1.tile([128, MS], f32, tag="ps1", name="ps1t")
            cwarm = nc.const_aps.scalar_like(1.0, ps_junk[:, :])
            cbr = cwarm.broadcast_to([128, MS])
            nc.tensor.matmul(ps_junk[:1, :], cwarm, cbr, start=True, stop=True)
            wwarm = cst.tile([128, 128], bf16)
            xwarm = cst.tile([128, 512], bf16)
            nc.vector.memset(wwarm[:, :], 0.01)
            nc.vector.memset(xwarm[:, :], 0.5)
            for _ in range(N_WARM):
                ps_junk = ps1.tile([128, MS], f32, tag="ps1", name="ps1t")
                nc.tensor.matmul(ps_junk[:, :], wwarm[:, :], xwarm[:, :],
                                 start=True, stop=True)

            b1_sb = cst.tile([128, INTER // 128], f32)
            b2_sb = cst.tile([128, HIDDEN // 128], f32)

            for mc in range(N_MC):
                m0 = mc * M_CHUNK
                # x chunk: [HIDDEN, M_CHUNK] -> [128, KT1 * M_CHUNK].
                # Split per k-tile so the first GEMM1 matmuls can start as
                # soon as k-tile 0 lands (subtile deps) instead of after the
                # whole 2.1 MB chunk.
                xt_sb = xp.tile([128, KT1 * M_CHUNK], bf16, tag="xt")
                if mc != 0:
                    # Non-first chunk: one bulk prefetch queued behind
                    # current work.
                    nc.sync.dma_start(
                        xt_sb[:, :].rearrange("p (a m) -> p a m", m=M_CHUNK),
                        xT[:, m0:m0 + M_CHUNK].rearrange(
                            "(a p) m -> p a m", p=128),
                    )
                # output accumulator: [HIDDEN, M_CHUNK] -> [128, N_HT * M_CHUNK]
                h2_sb = hp.tile([128, N_HT * M_CHUNK], f32, tag="h2")

                for pair in range(N_PAIR):
                    h1_half = []
                    w2_half = []
                    deferred_w2 = []
                    for half in range(2):
                        oc = pair * 2 + half
                        o0 = oc * O_CHUNK
                        cold = mc == 0 and pair == 0 and half == 0
                        # W1T slice [HIDDEN, O_CHUNK] -> [128, KT1 * O_CHUNK]
                        w1_sb = w1p.tile([128, KT1 * O_CHUNK], bf16, tag="w1")
                        if cold:
                            # Cold fill, spread across THREE dma-capable
                            # queues (sync/scalar/gpsimd).  Each queue leads
                            # with a TINY pilot transfer (the biases + a
                            # bias re-load) so the several-us first-transfer
                            # DMA spin-up is paid on 4-16KB instead of a
                            # 128KB tile the PE is about to need.  The
                            # (W1 k, x k ms0) tiles then round-robin across
                            # queues in the exact order the k-outer GEMM1
                            # pass consumes them; the ms1 x half trails.
                            def w1_args(k):
                                return (
                                    w1_sb[:, k * O_CHUNK:(k + 1) * O_CHUNK],
                                    w1T[k * 128:(k + 1) * 128, o0:o0 + O_CHUNK],
                                )

                            def xt_half(k, ms):
                                return (
                                    xt_sb[:, k * M_CHUNK + ms * MS:
                                          k * M_CHUNK + (ms + 1) * MS],
                                    xT[k * 128:(k + 1) * 128,
                                       m0 + ms * MS:m0 + (ms + 1) * MS],
                                )
                            pilot_sb = cst.tile([128, HIDDEN // 128], f32)
                            nc.sync.dma_start(b2_sb[:, :], b2[:, :])
                            nc.scalar.dma_start(b1_sb[:, :], b1[:, :])
                            nc.gpsimd.dma_start(pilot_sb[:, :], b2[:, :])
                            queues = [nc.gpsimd, nc.sync, nc.scalar]
                            need = []
                            for k in range(KT1):
                                need.append(w1_args(k))
                                need.append(xt_half(k, 0))
                            for k in range(KT1):
                                need.append(xt_half(k, 1))
                            for j, args in enumerate(need):
                                queues[j % 3].dma_start(*args)
                        else:
                            nc.sync.dma_start(
                                w1_sb[:, :].rearrange("p (a o) -> p a o",
                                                      o=O_CHUNK),
                                w1T[:, o0:o0 + O_CHUNK].rearrange(
                                    "(a p) o -> p a o", p=128),
                            )
                        # The cold half's W2 slice queues here, AFTER this
                        # half's W1 — W1(oc1) is needed ~15us before
                        # W2(oc0), and the sync queue delivers in FIFO
                        # order.
                        for args in deferred_w2:
                            nc.sync.dma_start(*args)
                        deferred_w2 = []

                        # GEMM1: h1T[o0:o0+512, m-chunk] in bf16
                        h1_sb = h1p.tile([128, N_OT * M_CHUNK], bf16, tag="h1")
                        if cold:
                            # k-outer order: consume k-tiles as they arrive.
                            # ms outer keeps live PSUM groups at N_OT = 4.
                            for ms in range(N_MS):
                                accs = [ps1.tile([128, MS], f32, tag="ps1",
                                                 name="ps1t")
                                        for _ in range(N_OT)]
                                for k in range(KT1):
                                    for ot in range(N_OT):
                                        nc.tensor.matmul(
                                            accs[ot][:, :],
                                            w1_sb[:, k * O_CHUNK + ot * 128:
                                                  k * O_CHUNK + (ot + 1) * 128],
                                            xt_sb[:, k * M_CHUNK + ms * MS:
                                                  k * M_CHUNK + (ms + 1) * MS],
                                            start=(k == 0),
                                            stop=(k == KT1 - 1),
                                        )
                                for ot in range(N_OT):
                                    nc.scalar.activation(
                                        h1_sb[:, ot * M_CHUNK + ms * MS:
                                              ot * M_CHUNK + (ms + 1) * MS],
                                        accs[ot][:, :],
                                        ID,
                                        bias=b1_sb[:, oc * N_OT + ot:
                                                   oc * N_OT + ot + 1],
                                        scale=1.0,
                                    )
                        else:
                            for ot in range(N_OT):
                                accs = [ps1.tile([128, MS], f32, tag="ps1",
                                                 name="ps1t")
                                        for _ in range(N_MS)]
                                for k in range(KT1):
                                    lhsT = w1_sb[:, k * O_CHUNK + ot * 128:
                                                 k * O_CHUNK + (ot + 1) * 128]
                                    for ms in range(N_MS):
                                        nc.tensor.matmul(
                                            accs[ms][:, :],
                                            lhsT,
                                            xt_sb[:, k * M_CHUNK + ms * MS:
                                                  k * M_CHUNK + (ms + 1) * MS],
                                            start=(k == 0),
                                            stop=(k == KT1 - 1),
                                        )
                                for ms in range(N_MS):
                                    nc.scalar.activation(
                                        h1_sb[:, ot * M_CHUNK + ms * MS:
                                              ot * M_CHUNK + (ms + 1) * MS],
                                        accs[ms][:, :],
                                        ID,
                                        bias=b1_sb[:, oc * N_OT + ot:
                                                   oc * N_OT + ot + 1],
                                        scale=1.0,
                                    )
                        h1_half.append(h1_sb)

                        # W2T slice [O_CHUNK, HIDDEN] -> [128, N_OT * HIDDEN].
                        # Emitted after GEMM1 so its DMA queues behind the
                        # critical-path x/W1 loads.
                        w2_sb = w2p.tile([128, N_OT * HIDDEN], bf16, tag="w2")
                        w2_args = (
                            w2_sb[:, :].rearrange("p (a n) -> p a n", n=HIDDEN),
                            w2T[o0:o0 + O_CHUNK, :].rearrange(
                                "(a p) n -> p a n", p=128),
                        )
                        if cold:
                            deferred_w2.append(w2_args)
                        else:
                            nc.sync.dma_start(*w2_args)
                        w2_half.append(w2_sb)

                    # GEMM2 for the o-pair: accumulate 8 k-steps in PSUM,
                    # then fold into h2_sb.
                    # On the final pair of the final m-chunk the k-loop runs
                    # ms-split (all ms0 steps, fold, half-DMA, then ms1) so
                    # the last output transfers overlap the remaining
                    # matmuls and the tail after the last matmul is just one
                    # 256KB half-DMA + fold.
                    tail_pair = pair == N_PAIR - 1 and mc == N_MC - 1
                    for ht in range(N_HT):
                        accs = [ps2.tile([128, MS], f32, tag="ps2", name="ps2t")
                                for _ in range(N_MS)]
                        ms_groups = ([[0, 1]] if not tail_pair
                                     else [[0], [1]])
                        for ms_grp in ms_groups:
                            for k in range(KT2):
                                half, ot = divmod(k, N_OT)
                                lhsT = w2_half[half][:, ot * HIDDEN + ht * 128:
                                                     ot * HIDDEN + (ht + 1) * 128]
                                for ms in ms_grp:
                                    nc.tensor.matmul(
                                        accs[ms][:, :],
                                        lhsT,
                                        h1_half[half][:, ot * M_CHUNK + ms * MS:
                                                      ot * M_CHUNK + (ms + 1) * MS],
                                        start=(k == 0),
                                        stop=(k == KT2 - 1),
                                    )
                            for ms in ms_grp:
                                dst = h2_sb[:, ht * M_CHUNK + ms * MS:
                                            ht * M_CHUNK + (ms + 1) * MS]
                                if pair == 0:
                                    nc.scalar.activation(
                                        dst, accs[ms][:, :], ID,
                                        bias=b2_sb[:, ht:ht + 1], scale=1.0,
                                    )
                                else:
                                    nc.vector.tensor_add(dst, dst,
                                                         accs[ms][:, :])
                                if tail_pair:
                                    # per-half output DMA, alternating
                                    # queues; the very last block further
                                    # splits each half into two 128KB
                                    # quarter-DMAs on parallel queues.
                                    if ht == N_HT - 1:
                                        qs = ([nc.sync, nc.gpsimd] if ms == 0
                                              else [nc.scalar, nc.gpsimd])
                                        for qi, q in enumerate(qs):
                                            c0 = ms * MS + qi * (MS // 2)
                                            q.dma_start(
                                                outT[ht * 128:(ht + 1) * 128,
                                                     m0 + c0:
                                                     m0 + c0 + MS // 2],
                                                h2_sb[:, ht * M_CHUNK + c0:
                                                      ht * M_CHUNK + c0
                                                      + MS // 2],
                                            )
                                    else:
                                        q = nc.sync if ms == 0 else nc.scalar
                                        q.dma_start(
                                            outT[ht * 128:(ht + 1) * 128,
                                                 m0 + ms * MS:
                                                 m0 + (ms + 1) * MS],
                                            h2_sb[:, ht * M_CHUNK + ms * MS:
                                                  ht * M_CHUNK
                                                  + (ms + 1) * MS],
                                        )
                        if pair == N_PAIR - 1 and not tail_pair:
                            # Stream each hout-row-block out as soon as its
                            # last fold lands — keeps the kernel tail short.
                            nc.sync.dma_start(
                                outT[ht * 128:(ht + 1) * 128,
                                     m0:m0 + M_CHUNK],
                                h2_sb[:, ht * M_CHUNK:(ht + 1) * M_CHUNK],
                            )

    nc.compile()
    _NC_CACHE = nc
    return nc


def _prep_core_inputs(x, W1, b1, W2, b2, i):
    bf = np.dtype("bfloat16") if hasattr(np, "bfloat16") else None
    import ml_dtypes
    bf = ml_dtypes.bfloat16
    return {
        "xT": np.ascontiguousarray(
            np.asarray(x[i], dtype=np.float32).T).astype(bf),
        "w1T": np.ascontiguousarray(
            np.asarray(W1[i], dtype=np.float32).T).astype(bf),
        "b1": np.ascontiguousarray(
            np.asarray(b1[i], dtype=np.float32).reshape(INTER // 128, 128).T),
        "w2T": np.ascontiguousarray(
            np.asarray(W2[i], dtype=np.float32).T).astype(bf),
        "b2": np.ascontiguousarray(
            np.asarray(b2[i], dtype=np.float32).reshape(HIDDEN // 128, 128).T),
    }


def kernel(x, W1, b1, W2, b2, _trace=False, _trace_kwargs=None):
    x = np.asarray(x, dtype=np.float32)
    orig_shape = x.shape
    xg = x.reshape(NUM_GEMMS, M, HIDDEN)

    nc = build_nc()
    in_maps = [_prep_core_inputs(xg, W1, b1, W2, b2, i) for i in range(NUM_GEMMS)]
    res = None
    for attempt in range(3):
        try:
            res = run_bass_kernel_spmd(
                nc, in_maps, list(range(NUM_GEMMS)),
                trace=_trace, **(_trace_kwargs or {}),
            )
            break
        except Exception:
            # transient NRT_EXEC_UNIT_UNRECOVERABLE has been observed on
            # rapid repeated runs; a short pause and retry recovers
            if attempt == 2:
                raise
            import time
            time.sleep(20)
    out = np.stack(
        [res.results[i]["outT"].T for i in range(NUM_GEMMS)], axis=0
    ).reshape(orig_shape).astype(np.float32)
    if _trace:
        return out, res
    return out


# revision 10
# speedup vs baseline: 1.0670x; 1.0014x over previous
"""Grouped-GEMM MoE expert MLP kernel for 8 Trainium2 NeuronCores.

Problem: x [8, 2048, 1024] f32, per-group W1 [8, 4096, 1024], b1 [8, 4096],
W2 [8, 1024, 4096], b2 [8, 1024] (torch Linear convention, y = x @ W.T + b):
  h1 = xg @ W1.T + b1        (per group)
  h2 = h1 @ W2.T + b2
Expert-parallel: core i owns group i entirely — no collectives.

Formulation is fully transposed so every DMA is contiguous and biases land on
the partition axis:
  h1T[o, m]   = matmul(lhsT=W1T[h,o] tiles, rhs=xT[h,m] tiles)  + b1[o]
  outT[ho, m] = matmul(lhsT=W2T[o,ho] tiles, rhs=h1T[o,m] tiles) + b2[ho]
(out = lhsT.T @ rhs contracts the partition axis of both operands.)
Host pre-transposes x/W1/W2 per shard and un-transposes the output.

Matmuls run in bfloat16 with fp32 PSUM accumulation and f32 biases.
Measured HW cadence: bf16 matmul [128k x 512f] = 215.8 ns back-to-back vs
fp32r's 226.7 ns (fp32r pays ~32 extra cycles per instruction for the 4-byte
weight load) — 2048 matmuls/core -> ~442 us PE floor.  End-to-end rel err
(vs f32 reference) ~3e-3, dominated by bf16 input rounding.

Per-core loop structure: 2 m-chunks of 1024 tokens; inside, 8 o-chunks of 512.
GEMM1 for an o-chunk feeds SBUF bf16 tiles h1T; GEMM2 accumulates PSUM over an
o-PAIR (1024, 8 k-steps) then folds into an f32 SBUF accumulator (first pair
via ScalarE copy+bias, later pairs via VectorE add) to keep PSUM pressure at
4+4 banks. Weights are streamed per m-chunk (2 x 16.8 MB bf16), x and out once.
"""
import sys

sys.path.insert(0, "/opt/trn_rl_repo")

import numpy as np

import concourse.bass as bass  # noqa: F401  (bass import initializes mybir deps)
import concourse.mybir as mybir
import concourse.tile as tile
from concourse import bacc
from concourse.bass_utils import run_bass_kernel_spmd

NUM_GEMMS = 8
HIDDEN = 1024
INTER = 4096
M = 2048  # tokens per group

M_CHUNK = 1024  # tokens per chunk (2 chunks)
MS = 512        # matmul moving free dim (one PSUM bank)
O_CHUNK = 512   # GEMM1 / weight-DMA granularity along INTER
O_PAIR = 1024   # GEMM2 PSUM accumulation span along INTER (8 k-steps)

f32 = mybir.dt.float32
bf16 = mybir.dt.bfloat16

N_MC = M // M_CHUNK              # 2
N_PAIR = INTER // O_PAIR         # 4
N_MS = M_CHUNK // MS             # 2
KT1 = HIDDEN // 128              # 8 k-tiles for GEMM1
KT2 = O_PAIR // 128              # 8 k-tiles per GEMM2 psum group
N_OT = O_CHUNK // 128            # 4 o-tiles per o-chunk
N_HT = HIDDEN // 128             # 8 hout-tiles
N_WARM = 6                       # bf16 warmup matmuls (HAM clock ramp)

_NC_CACHE = None


def build_nc():
    """Build + compile the single-core program (same on all 8 cores)."""
    global _NC_CACHE
    if _NC_CACHE is not None:
        return _NC_CACHE

    nc = bacc.Bacc("TRN2", target_bir_lowering=False, debug=False, num_devices=8)
    xT = nc.dram_tensor("xT", [HIDDEN, M], bf16, kind="ExternalInput").ap()
    w1T = nc.dram_tensor("w1T", [HIDDEN, INTER], bf16, kind="ExternalInput").ap()
    b1 = nc.dram_tensor("b1", [128, INTER // 128], f32, kind="ExternalInput").ap()
    w2T = nc.dram_tensor("w2T", [INTER, HIDDEN], bf16, kind="ExternalInput").ap()
    b2 = nc.dram_tensor("b2", [128, HIDDEN // 128], f32, kind="ExternalInput").ap()
    outT = nc.dram_tensor("outT", [HIDDEN, M], f32, kind="ExternalOutput").ap()

    ID = mybir.ActivationFunctionType.Identity

    with tile.TileContext(nc) as tc:
        with (
            tc.tile_pool(name="cst", bufs=1) as cst,
            tc.tile_pool(name="xp", bufs=1) as xp,
            tc.tile_pool(name="hp", bufs=1) as hp,
            tc.tile_pool(name="w1p", bufs=3) as w1p,
            tc.tile_pool(name="w2p", bufs=2) as w2p,
            tc.tile_pool(name="h1p", bufs=2) as h1p,
            tc.tile_pool(name="ps1", bufs=4, space="PSUM") as ps1,
            tc.tile_pool(name="ps2", bufs=4, space="PSUM") as ps2,
        ):
            # PE warmup while the first DMAs fill: starts the HAM clock
            # throttle release window (4/8 -> 8/8 after ~5.9us of gapless PE
            # activity) as early as possible.  First a const-operand fp32
            # matmul (available the moment the Tensor queue clears its
            # preamble ~7.3us, runs ~2.2us at the throttled clock), then a
            # few bf16 matmuls on DVE-memset tiles to bridge until the first
            # weight/x tiles land (~11us).  Real matmuls take over at half
            # clock for ~2us until the release at ~13.2us.
            ps_junk = ps1.tile([128, MS], f32, tag="ps1", name="ps1t")
            cwarm = nc.const_aps.scalar_like(1.0, ps_junk[:, :])
            cbr = cwarm.broadcast_to([128, MS])
            nc.tensor.matmul(ps_junk[:1, :], cwarm, cbr, start=True, stop=True)
            wwarm = cst.tile([128, 128], bf16)
            xwarm = cst.tile([128, 512], bf16)
            nc.vector.memset(wwarm[:, :], 0.01)
            nc.vector.memset(xwarm[:, :], 0.5)
            for _ in range(N_WARM):
                ps_junk = ps1.tile([128, MS], f32, tag="ps1", name="ps1t")
                nc.tensor.matmul(ps_junk[:, :], wwarm[:, :], xwarm[:, :],
                                 start=True, stop=True)

            b1_sb = cst.tile([128, INTER // 128], f32)
            b2_sb = cst.tile([128, HIDDEN // 128], f32)

            for mc in range(N_MC):
                m0 = mc * M_CHUNK
                # x chunk: [HIDDEN, M_CHUNK] -> [128, KT1 * M_CHUNK].
                # Split per k-tile so the first GEMM1 matmuls can start as
                # soon as k-tile 0 lands (subtile deps) instead of after the
                # whole 2.1 MB chunk.
                xt_sb = xp.tile([128, KT1 * M_CHUNK], bf16, tag="xt")
                if mc != 0:
                    # Non-first chunk: one bulk prefetch queued behind
                    # current work.
                    nc.sync.dma_start(
                        xt_sb[:, :].rearrange("p (a m) -> p a m", m=M_CHUNK),
                        xT[:, m0:m0 + M_CHUNK].rearrange(
                            "(a p) m -> p a m", p=128),
                    )
                # output accumulator: [HIDDEN, M_CHUNK] -> [128, N_HT * M_CHUNK]
                h2_sb = hp.tile([128, N_HT * M_CHUNK], f32, tag="h2")

                for pair in range(N_PAIR):
                    h1_half = []
                    w2_half = []
                    deferred_w2 = []
                    for half in range(2):
                        oc = pair * 2 + half
                        o0 = oc * O_CHUNK
                        cold = mc == 0 and pair == 0 and half == 0
                        # W1T slice [HIDDEN, O_CHUNK] -> [128, KT1 * O_CHUNK]
                        w1_sb = w1p.tile([128, KT1 * O_CHUNK], bf16, tag="w1")
                        if cold:
                            # Cold fill, spread across THREE dma-capable
                            # queues (sync/scalar/gpsimd).  Each queue leads
                            # with a TINY pilot transfer (the biases + a
                            # bias re-load) so the several-us first-transfer
                            # DMA spin-up is paid on 4-16KB instead of a
                            # 128KB tile the PE is about to need.  The
                            # (W1 k, x k ms0) tiles then round-robin across
                            # queues in the exact order the k-outer GEMM1
                            # pass consumes them; the ms1 x half trails.
                            def w1_args(k):
                                return (
                                    w1_sb[:, k * O_CHUNK:(k + 1) * O_CHUNK],
                                    w1T[k * 128:(k + 1) * 128, o0:o0 + O_CHUNK],
                                )

                            def xt_half(k, ms):
                                return (
                                    xt_sb[:, k * M_CHUNK + ms * MS:
                                          k * M_CHUNK + (ms + 1) * MS],
                                    xT[k * 128:(k + 1) * 128,
                                       m0 + ms * MS:m0 + (ms + 1) * MS],
                                )
                            pilot_sb = cst.tile([128, HIDDEN // 128], f32)
                            nc.sync.dma_start(b2_sb[:, :], b2[:, :])
                            nc.scalar.dma_start(b1_sb[:, :], b1[:, :])
                            nc.gpsimd.dma_start(pilot_sb[:, :], b2[:, :])
                            queues = [nc.gpsimd, nc.sync, nc.scalar]
                            need = []
                            for k in range(KT1):
                                need.append(w1_args(k))
                                need.append(xt_half(k, 0))
                            for k in range(KT1):
                                need.append(xt_half(k, 1))
                            for j, args in enumerate(need):
                                queues[j % 3].dma_start(*args)
                        else:
                            nc.sync.dma_start(
                                w1_sb[:, :].rearrange("p (a o) -> p a o",
                                                      o=O_CHUNK),
                                w1T[:, o0:o0 + O_CHUNK].rearrange(
                                    "(a p) o -> p a o", p=128),
                            )
                        # The cold half's W2 slice queues here, AFTER this
                        # half's W1 — W1(oc1) is needed ~15us before
                        # W2(oc0), and the sync queue delivers in FIFO
                        # order.
                        for args in deferred_w2:
                            nc.sync.dma_start(*args)
                        deferred_w2 = []

                        # GEMM1: h1T[o0:o0+512, m-chunk] in bf16
                        h1_sb = h1p.tile([128, N_OT * M_CHUNK], bf16, tag="h1")
                        if cold:
                            # k-outer order: consume k-tiles as they arrive.
                            # ms outer keeps live PSUM groups at N_OT = 4.
                            for ms in range(N_MS):
                                accs = [ps1.tile([128, MS], f32, tag="ps1",
                                                 name="ps1t")
                                        for _ in range(N_OT)]
                                for k in range(KT1):
                                    for ot in range(N_OT):
                                        nc.tensor.matmul(
                                            accs[ot][:, :],
                                            w1_sb[:, k * O_CHUNK + ot * 128:
                                                  k * O_CHUNK + (ot + 1) * 128],
                                            xt_sb[:, k * M_CHUNK + ms * MS:
                                                  k * M_CHUNK + (ms + 1) * MS],
                                            start=(k == 0),
                                            stop=(k == KT1 - 1),
                                        )
                                for ot in range(N_OT):
                                    nc.scalar.activation(
                                        h1_sb[:, ot * M_CHUNK + ms * MS:
                                              ot * M_CHUNK + (ms + 1) * MS],
                                        accs[ot][:, :],
                                        ID,
                                        bias=b1_sb[:, oc * N_OT + ot:
                                                   oc * N_OT + ot + 1],
                                        scale=1.0,
                                    )
                        else:
                            for ot in range(N_OT):
                                accs = [ps1.tile([128, MS], f32, tag="ps1",
                                                 name="ps1t")
                                        for _ in range(N_MS)]
                                for k in range(KT1):
                                    lhsT = w1_sb[:, k * O_CHUNK + ot * 128:
                                                 k * O_CHUNK + (ot + 1) * 128]
                                    for ms in range(N_MS):
                                        nc.tensor.matmul(
                                            accs[ms][:, :],
                                            lhsT,
                                            xt_sb[:, k * M_CHUNK + ms * MS:
                                                  k * M_CHUNK + (ms + 1) * MS],
                                            start=(k == 0),
                                            stop=(k == KT1 - 1),
                                        )
                                for ms in range(N_MS):
                                    nc.scalar.activation(
                                        h1_sb[:, ot * M_CHUNK + ms * MS:
                                              ot * M_CHUNK + (ms + 1) * MS],
                                        accs[ms][:, :],
                                        ID,
                                        bias=b1_sb[:, oc * N_OT + ot:
                                                   oc * N_OT + ot + 1],
                                        scale=1.0,
                                    )
                        h1_half.append(h1_sb)

                        # W2T slice [O_CHUNK, HIDDEN] -> [128, N_OT * HIDDEN].
                        # Emitted after GEMM1 so its DMA queues behind the
                        # critical-path x/W1 loads.
                        w2_sb = w2p.tile([128, N_OT * HIDDEN], bf16, tag="w2")
                        w2_args = (
                            w2_sb[:, :].rearrange("p (a n) -> p a n", n=HIDDEN),
                            w2T[o0:o0 + O_CHUNK, :].rearrange(
                                "(a p) n -> p a n", p=128),
                        )
                        if cold:
                            deferred_w2.append(w2_args)
                        else:
                            nc.sync.dma_start(*w2_args)
                        w2_half.append(w2_sb)

                    # GEMM2 for the o-pair: accumulate 8 k-steps in PSUM,
                    # then fold into h2_sb.
                    # On the final pair of the final m-chunk the k-loop runs
                    # ms-split (all ms0 steps, fold, half-DMA, then ms1) so
                    # the last output transfers overlap the remaining
                    # matmuls and the tail after the last matmul is just one
                    # 256KB half-DMA + fold.
                    tail_pair = pair == N_PAIR - 1 and mc == N_MC - 1
                    for ht in range(N_HT):
                        accs = [ps2.tile([128, MS], f32, tag="ps2", name="ps2t")
                                for _ in range(N_MS)]
                        ms_groups = ([[0, 1]] if not tail_pair
                                     else [[0], [1]])
                        for ms_grp in ms_groups:
                            for k in range(KT2):
                                half, ot = divmod(k, N_OT)
                                lhsT = w2_half[half][:, ot * HIDDEN + ht * 128:
                                                     ot * HIDDEN + (ht + 1) * 128]
                                for ms in ms_grp:
                                    nc.tensor.matmul(
                                        accs[ms][:, :],
                                        lhsT,
                                        h1_half[half][:, ot * M_CHUNK + ms * MS:
                                                      ot * M_CHUNK + (ms + 1) * MS],
                                        start=(k == 0),
                                        stop=(k == KT2 - 1),
                                    )
                            for ms in ms_grp:
                                dst = h2_sb[:, ht * M_CHUNK + ms * MS:
                                            ht * M_CHUNK + (ms + 1) * MS]
                                if pair == 0:
                                    nc.scalar.activation(
                                        dst, accs[ms][:, :], ID,
                                        bias=b2_sb[:, ht:ht + 1], scale=1.0,
                                    )
                                else:
                                    nc.vector.tensor_add(dst, dst,
                                                         accs[ms][:, :])
                                if tail_pair:
                                    # per-half output DMA, alternating
                                    # queues; the very last block further
                                    # splits each half into two 128KB
                                    # quarter-DMAs on parallel queues.
                                    # gpsimd stays out of the tail: its
                                    # queue epilogue DRAIN is ~2.9us and
                                    # must not run after the last transfer.
                                    if ht == N_HT - 1:
                                        qs = ([nc.sync, nc.scalar] if ms == 0
                                              else [nc.scalar, nc.sync])
                                        for qi, q in enumerate(qs):
                                            c0 = ms * MS + qi * (MS // 2)
                                            q.dma_start(
                                                outT[ht * 128:(ht + 1) * 128,
                                                     m0 + c0:
                                                     m0 + c0 + MS // 2],
                                                h2_sb[:, ht * M_CHUNK + c0:
                                                      ht * M_CHUNK + c0
                                                      + MS // 2],
                                            )
                                    else:
                                        q = nc.sync if ms == 0 else nc.scalar
                                        q.dma_start(
                                            outT[ht * 128:(ht + 1) * 128,
                                                 m0 + ms * MS:
                                                 m0 + (ms + 1) * MS],
                                            h2_sb[:, ht * M_CHUNK + ms * MS:
                                                  ht * M_CHUNK
                                                  + (ms + 1) * MS],
                                        )
                        if pair == N_PAIR - 1 and not tail_pair:
                            # Stream each hout-row-block out as soon as its
                            # last fold lands — keeps the kernel tail short.
                            nc.sync.dma_start(
                                outT[ht * 128:(ht + 1) * 128,
                                     m0:m0 + M_CHUNK],
                                h2_sb[:, ht * M_CHUNK:(ht + 1) * M_CHUNK],
                            )

    nc.compile()
    _NC_CACHE = nc
    return nc


def _prep_core_inputs(x, W1, b1, W2, b2, i):
    bf = np.dtype("bfloat16") if hasattr(np, "bfloat16") else None
    import ml_dtypes
    bf = ml_dtypes.bfloat16
    return {
        "xT": np.ascontiguousarray(
            np.asarray(x[i], dtype=np.float32).T).astype(bf),
        "w1T": np.ascontiguousarray(
            np.asarray(W1[i], dtype=np.float32).T).astype(bf),
        "b1": np.ascontiguousarray(
            np.asarray(b1[i], dtype=np.float32).reshape(INTER // 128, 128).T),
        "w2T": np.ascontiguousarray(
            np.asarray(W2[i], dtype=np.float32).T).astype(bf),
        "b2": np.ascontiguousarray(
            np.asarray(b2[i], dtype=np.float32).reshape(HIDDEN // 128, 128).T),
    }


def kernel(x, W1, b1, W2, b2, _trace=False, _trace_kwargs=None):
    x = np.asarray(x, dtype=np.float32)
    orig_shape = x.shape
    xg = x.reshape(NUM_GEMMS, M, HIDDEN)

    nc = build_nc()
    in_maps = [_prep_core_inputs(xg, W1, b1, W2, b2, i) for i in range(NUM_GEMMS)]
    res = None
    for attempt in range(3):
        try:
            res = run_bass_kernel_spmd(
                nc, in_maps, list(range(NUM_GEMMS)),
                trace=_trace, **(_trace_kwargs or {}),
            )
            break
        except Exception:
            # transient NRT_EXEC_UNIT_UNRECOVERABLE has been observed on
            # rapid repeated runs; a short pause and retry recovers
            if attempt == 2:
                raise
            import time
            time.sleep(20)
    out = np.stack(
        [res.results[i]["outT"].T for i in range(NUM_GEMMS)], axis=0
    ).reshape(orig_shape).astype(np.float32)
    if _trace:
        return out, res
    return out
